# revision 4
# baseline (speedup 1.0000x reference)
"""Multi-head attention (GQA prefill with KV cache) on 8 trn2 NeuronCores.

Sharding: tensor-parallel over heads. Core m owns KV head m (of 8) and the
4 query heads 4m..4m+3.  Each core computes its heads' attention output and
a partial x @ wo.T contribution; the host sums the 8 partials.

Layout notes:
  - All activations on device are "feature-major" ([feature, token]) so the
    token dim rides the matmul moving dim; x is transposed on the host once.
  - RoPE's (even, odd) pair interleave is removed by permuting wq/wk rows and
    cache_k's head_dim on the host (QK^T is invariant to a shared permutation
    of head_dim), so on device RoPE is plain 64-partition block arithmetic.
  - Scores are computed transposed ([key, query]) so the attention output
    lands feature-major, which feeds the wo matmul directly.
  - The attention inner loop is software-pipelined in a flat slot stream:
    score matmuls for slot p, AV matmuls for slot p-4, and the softmax
    normalization chain for the window finishing at slot p-6.  This keeps the
    in-order PE queue from stalling on the scalar-engine exp or the
    reciprocal.
  - The softmax denominator is built by folding exp tiles pairwise on the
    vector engine (bf16), then 4 accumulating ones-matmuls per window
    instead of 16.
"""

import os
import sys

import numpy as np

if "/opt/trn_rl_repo" not in sys.path:
    sys.path.insert(0, "/opt/trn_rl_repo")

import ml_dtypes

import concourse.bass as bass
import concourse.mybir as mybir
import concourse.tile as tile
from concourse.bass_utils import run_bass_kernel_spmd
from concourse.masks import make_identity

BF16 = mybir.dt.bfloat16
F32 = mybir.dt.float32
NP_BF16 = ml_dtypes.bfloat16

B, S, DIM = 4, 1024, 4096
N_HEADS, N_KV_HEADS = 32, 8
HD = 128
PAST = 1024
NCORES = 8
NQ = N_HEADS // NCORES  # 4 q heads per core
T = B * S  # 4096 tokens
DT = DIM // 128  # 32 contraction tiles
CH = 512  # projection token-chunk
NCH_B = S // CH  # chunks per batch (2)
SCW = 512  # attention query-chunk width
NKT = (PAST + S) // 128  # 16 key tiles per batch
NW = NQ * (S // SCW)  # attention windows per batch (8)
NKP = NKT // 2  # slots per window (8); each slot covers 2 key tiles
LAG_AV = 4  # slots between score and AV stages
LAG_NORM = 6  # slots between score stage and normalization chain
ISQRT_HD = 1.0 / float(np.sqrt(HD))

LAST_EXEC_NS = None
LAST_RESULTS = None

_CACHED = {}


def _split_multi_waits(nc):
    """walrus' per-instruction sync encoding fits one wait; hoist extras
    onto standalone EventSemaphore instructions on the same engine queue."""
    for f in nc.m.functions:
        for blk in f.blocks:
            insts = blk.instructions
            if not any(i.sync_info and i.sync_info.on_wait
                       and len(i.sync_info.on_wait) > 1 for i in insts):
                continue
            new = []
            for inst in insts:
                si = inst.sync_info
                if si is not None and si.on_wait and len(si.on_wait) > 1:
                    waits = list(si.on_wait)
                    for wt in waits[:-1]:
                        evs = mybir.InstEventSemaphore(
                            name=f"I-wsplit-{nc.next_id()}", ins=[], outs=[])
                        evs.engine = inst.engine
                        evs.sync_info = mybir.SyncInfo(on_wait=[wt],
                                                       on_update=[])
                        new.append(evs)
                    inst.sync_info = mybir.SyncInfo(
                        on_wait=[waits[-1]],
                        on_update=list(si.on_update or []))
                new.append(inst)
            insts[:] = new


def _build_nc(split_waits=True, mode="full"):
    nc = bass.Bass("TRN2", target_bir_lowering=False, debug=False,
                   num_devices=NCORES)

    xt = nc.dram_tensor("xt", [DIM, T], BF16, kind="ExternalInput")
    wqt = nc.dram_tensor("wqt", [DIM, NQ * HD], BF16, kind="ExternalInput")
    wkt = nc.dram_tensor("wkt", [DIM, HD], BF16, kind="ExternalInput")
    wvt = nc.dram_tensor("wvt", [DIM, HD], BF16, kind="ExternalInput")
    wot = nc.dram_tensor("wot", [NQ * HD, DIM], BF16, kind="ExternalInput")
    ckt = nc.dram_tensor("ckt", [B, HD, PAST], BF16, kind="ExternalInput")
    cv = nc.dram_tensor("cv", [B, PAST, HD], BF16, kind="ExternalInput")
    cos = nc.dram_tensor("cos", [HD // 2, S], BF16, kind="ExternalInput")
    sin = nc.dram_tensor("sin", [HD // 2, S], BF16, kind="ExternalInput")
    out_p = nc.dram_tensor("out_p", [T, DIM], BF16, kind="ExternalOutput")

    with tile.TileContext(nc) as tc:
        _emit(tc, nc, xt, wqt, wkt, wvt, wot, ckt, cv, cos, sin, out_p,
              mode=mode)
    if split_waits:
        _split_multi_waits(nc)
    return nc


def _emit(tc, nc, xt, wqt, wkt, wvt, wot, ckt, cv, cos, sin, out_p,
          mode="full"):
    from contextlib import ExitStack
    do_attn = mode in ("full", "bc")
    do_wo = mode in ("full", "bd")
    do_xdma = "nodma" not in mode
    do_rope = "norope" not in mode

    with ExitStack() as ctx:
        cw = ctx.enter_context(tc.tile_pool(name="consts", bufs=1))
        pb = ctx.enter_context(tc.tile_pool(name="perbatch", bufs=1))
        wk = ctx.enter_context(tc.tile_pool(name="work", bufs=2))
        ps = ctx.enter_context(tc.tile_pool(name="ps", bufs=3, space="PSUM"))

        # ---- resident constants -------------------------------------------
        # DMA order puts what the first matmuls need (cos/sin, wk, wv, the
        # first x chunk) ahead of wq/wo/caches on the sync queue.
        cos_sb = cw.tile([128, S], BF16, name="cos_sb")
        nc.sync.dma_start(out=cos_sb[0:64, :], in_=cos[:, :])
        nc.sync.dma_start(out=cos_sb[64:128, :], in_=cos[:, :])
        sin_sb = cw.tile([128, S], BF16, name="sin_sb")
        nc.sync.dma_start(out=sin_sb[0:64, :], in_=sin[:, :])
        nc.sync.dma_start(out=sin_sb[64:128, :], in_=sin[:, :])
        wkt_sb = cw.tile([128, DT * HD], BF16, name="wkt_sb")
        nc.sync.dma_start(
            out=wkt_sb.rearrange("p (n j) -> p n j", n=DT),
            in_=wkt[:, :].rearrange("(n p) j -> p n j", p=128))
        wvt_sb = cw.tile([128, DT * HD], BF16, name="wvt_sb")
        nc.sync.dma_start(
            out=wvt_sb.rearrange("p (n j) -> p n j", n=DT),
            in_=wvt[:, :].rearrange("(n p) j -> p n j", p=128))

        # Prefetch the first x chunk before the bigger weight loads so the
        # K-projection can start ~25us in.
        xt0_t = wk.tile([128, DT * CH], BF16, name="xt_t", tag="xt")
        if do_xdma:
            nc.sync.dma_start(
                out=xt0_t.rearrange("p (n t) -> p n t", n=DT),
                in_=xt[:, 0:CH].rearrange("(n p) t -> p n t", p=128))
        else:
            nc.gpsimd.memset(xt0_t, 0.0)

        wqt_sb = cw.tile([128, DT * NQ * HD], BF16, name="wqt_sb")
        nc.sync.dma_start(
            out=wqt_sb.rearrange("p (n j) -> p n j", n=DT),
            in_=wqt[:, :].rearrange("(n p) j -> p n j", p=128))
        wot_sb = cw.tile([128, NQ * DIM], BF16, name="wot_sb")
        nc.sync.dma_start(
            out=wot_sb.rearrange("p (n d) -> p n d", n=NQ),
            in_=wot[:, :].rearrange("(n p) d -> p n d", p=128))
        ones_sb = cw.tile([128, 1], BF16, name="ones_sb")
        nc.vector.memset(ones_sb, 1.0)
        onescol_sb = cw.tile([1, 128], BF16, name="onescol_sb")
        nc.vector.memset(onescol_sb, 1.0)
        ident = cw.tile([128, 128], BF16, name="ident")
        make_identity(nc, ident)

        def rope(dst_tile, dst_col, src_ps, cosc, sinc, n):
            """src layout (r|i) on partition halves.
            dst[0:64] = r*cos - i*sin ; dst[64:128] = r*sin + i*cos.
            Two full-width muls: tc = (r*cos | i*cos), ts = (r*sin | i*sin),
            then dst_r = tc_hi - ts_lo, dst_i = ts_hi + tc_lo."""
            if not do_rope:
                nc.vector.tensor_copy(dst_tile[:, dst_col:dst_col + n],
                                      src_ps)
                return
            tc_ = wk.tile([128, CH], F32, name="rope_tc", tag="rope_tc",
                          bufs=1)
            ts_ = wk.tile([128, CH], F32, name="rope_ts", tag="rope_ts",
                          bufs=1)
            nc.vector.tensor_mul(tc_[:, :n], src_ps, cosc)
            nc.vector.tensor_mul(ts_[0:64, :n], src_ps[64:128, :],
                                 sinc[64:128, :])
            nc.vector.tensor_mul(ts_[64:128, :n], src_ps[0:64, :],
                                 sinc[0:64, :])
            nc.vector.tensor_sub(dst_tile[0:64, dst_col:dst_col + n],
                                 tc_[0:64, :n], ts_[0:64, :n])
            nc.vector.tensor_add(dst_tile[64:128, dst_col:dst_col + n],
                                 ts_[64:128, :n], tc_[64:128, :n])

        for b in range(B):
            qb_t = pb.tile([128, NQ * S], BF16, name="qb", tag="qb")
            kb_t = pb.tile([128, S], BF16, name="kb", tag="kb")
            vb_t = pb.tile([128, S], BF16, name="vb", tag="vb")
            attnb_t = pb.tile([128, NQ * S], BF16, name="attnb", tag="attnb")

            # ---- projections + rope, per 512-token chunk ------------------
            for c in range(NCH_B):
                p0 = c * CH
                if b == 0 and c == 0:
                    xt_t = xt0_t
                else:
                    xt_t = wk.tile([128, DT * CH], BF16, name="xt_t",
                                   tag="xt")
                    if do_xdma:
                        nc.sync.dma_start(
                            out=xt_t.rearrange("p (n t) -> p n t", n=DT),
                            in_=xt[:, b * S + p0: b * S + p0 + CH].rearrange(
                                "(n p) t -> p n t", p=128))
                    else:
                        nc.gpsimd.memset(xt_t, 0.0)
                cosc = cos_sb[:, p0:p0 + CH]
                sinc = sin_sb[:, p0:p0 + CH]

                k_ps = ps.tile([128, CH], F32, name="k_ps", tag="acc", bufs=3)
                for d in range(DT):
                    nc.tensor.matmul(k_ps,
                                     lhsT=wkt_sb[:, d * HD:(d + 1) * HD],
                                     rhs=xt_t[:, d * CH:(d + 1) * CH],
                                     start=(d == 0), stop=(d == DT - 1))
                rope(kb_t, p0, k_ps, cosc, sinc, CH)

                v_ps = ps.tile([128, CH], F32, name="v_ps", tag="acc", bufs=3)
                for d in range(DT):
                    nc.tensor.matmul(v_ps,
                                     lhsT=wvt_sb[:, d * HD:(d + 1) * HD],
                                     rhs=xt_t[:, d * CH:(d + 1) * CH],
                                     start=(d == 0), stop=(d == DT - 1))
                vcp = wk.tile([128, CH], BF16, name="vcp", tag="vcp", bufs=1)
                nc.scalar.copy(vcp, v_ps)
                for tsub in range(CH // 128):
                    vtr_ps = ps.tile([128, 128], BF16, name="vtr_ps",
                                     tag="acc", bufs=3)
                    nc.tensor.transpose(vtr_ps,
                                        vcp[:, tsub * 128:(tsub + 1) * 128],
                                        ident)
                    col = (c * (CH // 128) + tsub) * 128
                    nc.vector.tensor_copy(vb_t[:, col:col + 128], vtr_ps)

                for j in range(NQ):
                    q_ps = ps.tile([128, CH], F32, name="q_ps", tag="acc",
                                   bufs=3)
                    for d in range(DT):
                        nc.tensor.matmul(
                            q_ps,
                            lhsT=wqt_sb[:, d * NQ * HD + j * HD:
                                        d * NQ * HD + (j + 1) * HD],
                            rhs=xt_t[:, d * CH:(d + 1) * CH],
                            start=(d == 0), stop=(d == DT - 1))
                    rope(qb_t, j * S + p0, q_ps, cosc, sinc, CH)

            # caches: needed only at attention; keep their DMAs behind the
            # first chunk loads on the sync queue.
            ckt_b = pb.tile([128, PAST], BF16, name="ckt_b", tag="ckt_b",
                            bufs=1)
            nc.sync.dma_start(out=ckt_b, in_=ckt[b, :, :])
            cv_b = pb.tile([128, PAST], BF16, name="cv_b", tag="cv_b",
                           bufs=1)
            nc.sync.dma_start(
                out=cv_b.rearrange("p (n d) -> p n d", n=PAST // 128),
                in_=cv[b, :, :].rearrange("(n p) d -> p n d", p=128))

            def k_lhsT(kt):
                if kt < PAST // 128:
                    return ckt_b[:, kt * 128:(kt + 1) * 128]
                kn = kt - PAST // 128
                return kb_t[:, kn * 128:(kn + 1) * 128]

            def v_lhsT(kt):
                if kt < PAST // 128:
                    return cv_b[:, kt * 128:(kt + 1) * 128]
                kn = kt - PAST // 128
                return vb_t[:, kn * 128:(kn + 1) * 128]

            # ---- attention: software-pipelined flat slot stream -----------
            # window w = h * (S//SCW) + sc ; slot p covers (w, kp) with
            # kp = p % NKP, key tiles (2kp, 2kp+1).
            if do_attn:
                nslots = NW * NKP
                exp_ts = {}   # slot -> exp tile [128, 2*SCW] bf16
                f_ts = {}     # slot -> folded [128, SCW] bf16
                g_ts = {}     # (w, i) -> pair tile [128, SCW] bf16
                out_ts = {}   # w -> out_ps [128, SCW] f32 PSUM
                sums_ts = {}  # w -> sums_ps [1, SCW] f32 PSUM
                inv_ts = {}   # w -> inv bf16 [1, SCW]

                for p in range(nslots + LAG_NORM + 2):
                    # stage 1: scores + exp + folds for slot p
                    if p < nslots:
                        w, kp = divmod(p, NKP)
                        h, sc = divmod(w, S // SCW)
                        s0 = sc * SCW
                        sc_ps = ps.tile([128, 2 * SCW], F32, name="sc_ps",
                                        tag="sc2", bufs=2)
                        for i in range(2):
                            nc.tensor.matmul(
                                sc_ps[:, i * SCW:(i + 1) * SCW],
                                lhsT=k_lhsT(2 * kp + i),
                                rhs=qb_t[:, h * S + s0:h * S + s0 + SCW])
                        exp_t = wk.tile([128, 2 * SCW], BF16, name="exp_t",
                                        tag="exp", bufs=6)
                        nc.scalar.activation(exp_t, sc_ps,
                                             mybir.ActivationFunctionType.Exp,
                                             scale=ISQRT_HD)
                        exp_ts[p] = exp_t
                        f_t = wk.tile([128, SCW], BF16, name="f_t", tag="f",
                                      bufs=3)
                        nc.vector.tensor_add(f_t, exp_t[:, 0:SCW],
                                             exp_t[:, SCW:2 * SCW])
                        f_ts[p] = f_t
                        if kp % 2 == 1:
                            g_t = wk.tile([128, SCW], BF16, name="g_t",
                                          tag="g", bufs=4)
                            nc.gpsimd.tensor_add(g_t, f_ts[p - 1], f_t)
                            g_ts[(w, kp // 2)] = g_t
                            del f_ts[p - 1], f_ts[p]

                    # stage 2: AV + denominator matmuls for slot p - LAG_AV
                    p2 = p - LAG_AV
                    if 0 <= p2 < nslots:
                        w2, kp2 = divmod(p2, NKP)
                        if kp2 == 0:
                            out_ts[w2] = ps.tile([128, SCW], F32,
                                                 name="out_ps", tag="acc",
                                                 bufs=3)
                        ex = exp_ts.pop(p2)
                        for i in range(2):
                            nc.tensor.matmul(
                                out_ts[w2], lhsT=v_lhsT(2 * kp2 + i),
                                rhs=ex[:, i * SCW:(i + 1) * SCW],
                                start=(kp2 == 0 and i == 0),
                                stop=(kp2 == NKP - 1 and i == 1))
                        if kp2 % 2 == 1:
                            if kp2 == 1:
                                sums_ts[w2] = ps.tile([1, SCW], F32,
                                                      name="sums_ps",
                                                      tag="sum", bufs=1)
                            nc.tensor.matmul(
                                sums_ts[w2], lhsT=ones_sb,
                                rhs=g_ts.pop((w2, kp2 // 2)),
                                start=(kp2 == 1), stop=(kp2 == NKP - 1))
                        if kp2 == NKP - 1:
                            inv_t = wk.tile([1, SCW], BF16, name="inv_t",
                                            tag="inv_t", bufs=2)
                            with nc.allow_low_precision(
                                    reason="softmax denom bcast via bf16"):
                                nc.vector.reciprocal(inv_t,
                                                     sums_ts.pop(w2))
                            inv_ts[w2] = inv_t

                    # stage 3: normalization for the window ending at
                    # slot p - LAG_NORM
                    p3 = p - LAG_NORM
                    if 0 <= p3 < nslots:
                        w3, kp3 = divmod(p3, NKP)
                        if kp3 == NKP - 1:
                            h3, sc3 = divmod(w3, S // SCW)
                            s3 = sc3 * SCW
                            inv_ps = ps.tile([128, SCW], F32, name="inv_ps",
                                             tag="acc", bufs=3)
                            nc.tensor.matmul(inv_ps, lhsT=onescol_sb,
                                             rhs=inv_ts.pop(w3))
                            inv_bc = wk.tile([128, SCW], BF16, name="inv_bc",
                                             tag="inv_bc", bufs=2)
                            nc.any.tensor_copy(inv_bc, inv_ps)
                            nc.vector.tensor_mul(
                                attnb_t[:, h3 * S + s3:h3 * S + s3 + SCW],
                                out_ts.pop(w3), inv_bc)

            # ---- output projection (partial) ------------------------------
            attn_src = attnb_t if do_attn else qb_t
            for tt in range(S // 128 if do_wo else 0):
                for dc in range(DIM // SCW):
                    wo_ps = ps.tile([128, SCW], F32, name="wo_ps", tag="acc",
                                    bufs=3)
                    for j in range(NQ):
                        nc.tensor.matmul(
                            wo_ps,
                            lhsT=attn_src[:, j * S + tt * 128:
                                          j * S + (tt + 1) * 128],
                            rhs=wot_sb[:, j * DIM + dc * SCW:
                                       j * DIM + (dc + 1) * SCW],
                            start=(j == 0), stop=(j == NQ - 1))
                    st = wk.tile([128, SCW], BF16, name="st", tag="st",
                                 bufs=3)
                    if dc % 2 == 0:
                        nc.scalar.copy(st, wo_ps)
                    else:
                        nc.vector.tensor_copy(st, wo_ps)
                    row = (b * (S // 128) + tt) * 128
                    nc.sync.dma_start(
                        out=out_p[row:row + 128, dc * SCW:(dc + 1) * SCW],
                        in_=st)


def _rope_perm():
    # even features first, then odd — per 128-wide head
    return np.concatenate([np.arange(0, HD, 2), np.arange(1, HD, 2)])


def _prep_inputs(x, freqs_cos, freqs_sin, cache_k, cache_v, wq, wk, wv, wo):
    perm = _rope_perm()
    xt = np.ascontiguousarray(
        x.reshape(T, DIM).T).astype(NP_BF16)
    cos_t = np.ascontiguousarray(freqs_cos.T).astype(NP_BF16)
    sin_t = np.ascontiguousarray(freqs_sin.T).astype(NP_BF16)

    in_maps = []
    for m in range(NCORES):
        wq_m = wq[m * NQ * HD:(m + 1) * NQ * HD]  # (512, 4096)
        wq_m = wq_m.reshape(NQ, HD, DIM)[:, perm, :].reshape(NQ * HD, DIM)
        wqt_m = np.ascontiguousarray(wq_m.T).astype(NP_BF16)
        wk_m = wk[m * HD:(m + 1) * HD][perm]
        wkt_m = np.ascontiguousarray(wk_m.T).astype(NP_BF16)
        wv_m = wv[m * HD:(m + 1) * HD]
        wvt_m = np.ascontiguousarray(wv_m.T).astype(NP_BF16)
        wot_m = np.ascontiguousarray(
            wo[:, m * NQ * HD:(m + 1) * NQ * HD].T).astype(NP_BF16)
        ckt_m = np.ascontiguousarray(
            cache_k[:, m][:, :, perm].transpose(0, 2, 1)).astype(NP_BF16)
        cv_m = np.ascontiguousarray(cache_v[:, m]).astype(NP_BF16)
        in_maps.append({
            "xt": xt, "wqt": wqt_m, "wkt": wkt_m, "wvt": wvt_m,
            "wot": wot_m, "ckt": ckt_m, "cv": cv_m,
            "cos": cos_t, "sin": sin_t,
        })
    return in_maps


def kernel(x, freqs_cos, freqs_sin, cache_k, cache_v, wq, wk, wv, wo):
    global LAST_EXEC_NS, LAST_RESULTS
    if "nc" not in _CACHED:
        _CACHED["nc"] = _build_nc()
    nc = _CACHED["nc"]

    in_maps = _prep_inputs(np.asarray(x), np.asarray(freqs_cos),
                           np.asarray(freqs_sin), np.asarray(cache_k),
                           np.asarray(cache_v), np.asarray(wq),
                           np.asarray(wk), np.asarray(wv), np.asarray(wo))

    trace = os.environ.get("KERNEL_TRACE", "0") == "1"
    try:
        res = run_bass_kernel_spmd(nc, in_maps, core_ids=list(range(NCORES)),
                                   trace=trace)
    except (ImportError, ModuleNotFoundError):
        # NTFF profiling hook unavailable in this environment
        res = run_bass_kernel_spmd(nc, in_maps, core_ids=list(range(NCORES)),
                                   trace=False)
    LAST_EXEC_NS = res.exec_time_ns
    LAST_RESULTS = res

    total = np.zeros((T, DIM), dtype=np.float64)
    for r in res.results:
        total += r["out_p"].astype(np.float64)
    return total.astype(np.float32).reshape(B, S, DIM)
